# revision 14
# baseline (speedup 1.0000x reference)
"""Bisection variant: EXACT baseline structure, bf16 matmul dtypes only."""

import numpy as np
import ml_dtypes

import concourse.bass as bass
import concourse.bacc as bacc
import concourse.mybir as mybir
import concourse.tile as tile
from concourse.bass_utils import run_bass_kernel_spmd

F32 = mybir.dt.float32
BF16 = mybir.dt.bfloat16
N_CORES = 8
S = 4096
D = 512
H = 8
DH = 64
HP = 66
S_BLK = S // N_CORES
NCH = S_BLK // 128
NHP = H // 2
EPS = 1e-6
W = NHP * HP

_CACHE = {}


def _build():
    _ctx = {}
    Alu = mybir.AluOpType
    Act = mybir.ActivationFunctionType
    nc = bacc.Bacc("TRN2", target_bir_lowering=False, debug=False,
                   num_devices=N_CORES)

    hst_d = nc.dram_tensor("hst", [D, S_BLK], BF16, kind="ExternalInput").ap()
    wq_d = nc.dram_tensor("wq", [D, D], BF16, kind="ExternalInput").ap()
    wk_d = nc.dram_tensor("wk", [D, D], BF16, kind="ExternalInput").ap()
    wv_d = nc.dram_tensor("wv", [D, D], BF16, kind="ExternalInput").ap()
    wo_d = nc.dram_tensor("wo", [D, D], BF16, kind="ExternalInput").ap()
    bqt_d = nc.dram_tensor("bqt", [128, NHP], F32, kind="ExternalInput").ap()
    bkt_d = nc.dram_tensor("bkt", [128, NHP], F32, kind="ExternalInput").ap()
    bv_d = nc.dram_tensor("bvr", [1, D], BF16, kind="ExternalInput").ap()
    mz_d = nc.dram_tensor("mz", [128, W], F32, kind="ExternalInput").ap()
    pm_d = nc.dram_tensor("pmask", [128, N_CORES], F32, kind="ExternalInput").ap()
    y_d = nc.dram_tensor("y", [S_BLK, D], F32, kind="ExternalOutput").ap()

    with tile.TileContext(nc) as tc:
        with (
            tc.tile_pool(name="const", bufs=1) as cpool,
            tc.tile_pool(name="wpool", bufs=1) as wpool,
            tc.tile_pool(name="data", bufs=1) as dpool,
            tc.tile_pool(name="tmp", bufs=3) as tpool,
            tc.tile_pool(name="small", bufs=4) as spool,
            tc.tile_pool(name="dram", bufs=1, space="DRAM") as drpool,
        ):
            hsT = [dpool.tile([128, S_BLK], BF16, name=f"hsT{i}")
                   for i in range(4)]
            for i in range(4):
                nc.sync.dma_start(hsT[i][:], hst_d[i * 128:(i + 1) * 128, :])
            wk_t = [wpool.tile([128, D], BF16, name=f"wk{i}") for i in range(4)]
            wv_t = [wpool.tile([128, D], BF16, name=f"wv{i}") for i in range(4)]
            wq_t = [wpool.tile([128, D], BF16, name=f"wq{i}") for i in range(4)]
            wo_t = [wpool.tile([128, D], BF16, name=f"wo{i}") for i in range(4)]
            for i in range(4):
                sl = slice(i * 128, (i + 1) * 128)
                nc.scalar.dma_start(wk_t[i][:], wk_d[sl, :])
                nc.scalar.dma_start(wv_t[i][:], wv_d[sl, :])
            bvr = cpool.tile([1, D], BF16)
            nc.sync.dma_start(bvr[:], bv_d[:])
            mz = cpool.tile([128, W], F32)
            nc.sync.dma_start(mz[:], mz_d[:])
            pmask = cpool.tile([128, N_CORES], F32)
            nc.sync.dma_start(pmask[:], pm_d[:])
            bqt = cpool.tile([128, NHP], F32)
            nc.sync.dma_start(bqt[:], bqt_d[:])
            bkt = cpool.tile([128, NHP], F32)
            nc.sync.dma_start(bkt[:], bkt_d[:])
            for i in range(4):
                sl = slice(i * 128, (i + 1) * 128)
                nc.scalar.dma_start(wq_t[i][:], wq_d[sl, :])
            for i in range(4):
                sl = slice(i * 128, (i + 1) * 128)
                nc.scalar.dma_start(wo_t[i][:], wo_d[sl, :])

            # ---- constants -------------------------------------------------
            ones128 = cpool.tile([128, 128], F32)
            nc.gpsimd.memset(ones128[:], 1.0)
            ident = cpool.tile([128, 128], F32)
            nc.gpsimd.affine_select(ident[:], ones128[:], pattern=[[1, 128]],
                                    compare_op=Alu.is_equal, fill=0.0, base=0,
                                    channel_multiplier=-1)
            ones128b = cpool.tile([128, 128], BF16)
            nc.gpsimd.memset(ones128b[:], 1.0)
            identb = cpool.tile([128, 128], BF16)
            nc.gpsimd.affine_select(identb[:], ones128b[:], pattern=[[1, 128]],
                                    compare_op=Alu.is_equal, fill=0.0, base=0,
                                    channel_multiplier=-1)
            triu2 = cpool.tile([128, 256], F32)
            for half in range(2):
                nc.gpsimd.affine_select(
                    triu2[:, half * 128:(half + 1) * 128], ones128[:],
                    pattern=[[1, 128]], compare_op=Alu.is_ge, fill=0.0,
                    base=0, channel_multiplier=-1)
            ones1 = cpool.tile([1, 128], BF16)
            nc.gpsimd.memset(ones1[:], 1.0)

            sk_row = [dpool.tile([128, D], BF16, name=f"skr{i}")
                      for i in range(4)]
            v_sb = [dpool.tile([128, H * HP], BF16, name=f"v{i}")
                    for i in range(4)]
            L_sb = [None] + [dpool.tile([128, W], F32, name=f"L{c}")
                             for c in range(1, NCH)]
            ball = dpool.tile([128, W], BF16, name="ball")

            with tc.tile_pool(name="ps1", bufs=1, space="PSUM") as ps1:
                # ---- phase A: kT proj -> elu -> transpose -> sk_row -------
                skT = [dpool.tile([128, S_BLK], BF16, name=f"skT{hp}")
                       for hp in range(NHP)]
                for hp in range(NHP):
                    psk = ps1.tile([128, S_BLK], F32, name="psbig", bufs=2)
                    for dt in range(4):
                        nc.tensor.matmul(
                            psk[:], wk_t[dt][:, hp * 128:(hp + 1) * 128],
                            hsT[dt][:], start=(dt == 0), stop=(dt == 3))
                    e_t = tpool.tile([128, S_BLK], F32, name="elu_e")
                    r_t = tpool.tile([128, S_BLK], F32, name="elu_r")
                    nc.scalar.activation(e_t[:], psk[:], Act.Exp,
                                         bias=bkt[:, hp:hp + 1])
                    nc.scalar.activation(r_t[:], psk[:], Act.Relu,
                                         bias=bkt[:, hp:hp + 1])
                    nc.vector.scalar_tensor_tensor(
                        skT[hp][:], e_t[:], 1.0, r_t[:],
                        op0=Alu.min, op1=Alu.add)
                    for c in range(NCH):
                        psTr = ps1.tile([128, 128], BF16, name="pstr", bufs=2,
                                        padded_shape=[128, 1024])
                        nc.tensor.transpose(
                            psTr[:], skT[hp][:, c * 128:(c + 1) * 128],
                            identb[:])
                        nc.vector.tensor_copy(
                            sk_row[c][:, hp * 128:(hp + 1) * 128], psTr[:])
                # ---- v proj + U block sums --------------------------------
                for st in range(4):
                    psv = ps1.tile([128, D], F32, name="psbig", bufs=2)
                    for dt in range(4):
                        nc.tensor.matmul(
                            psv[:], hsT[dt][:, st * 128:(st + 1) * 128],
                            wv_t[dt][:], start=(dt == 0), stop=False)
                    nc.tensor.matmul(psv[:], ones1[:], bvr[:],
                                     start=False, stop=True)
                    v3 = v_sb[st].rearrange("p (h e) -> p h e", e=HP)
                    nc.vector.tensor_copy(
                        v3[:, :, 0:DH],
                        psv.rearrange("p (h e) -> p h e", e=DH))
                    nc.gpsimd.memset(v3[:, :, DH:HP], 1.0)

                    # U for this chunk -> accumulate local prefix in SBUF
                    for hp in range(NHP):
                        psU = ps1.tile([128, HP], F32, name="psu", bufs=2,
                                       padded_shape=[128, 512])
                        for sub in range(2):
                            h = 2 * hp + sub
                            nc.tensor.matmul(
                                psU[sub * 64:(sub + 1) * 64, :],
                                sk_row[st][:, h * DH:(h + 1) * DH],
                                v_sb[st][:, h * HP:(h + 1) * HP],
                                start=True, stop=True,
                                tile_position=(0, 64 * sub))
                        dest = L_sb[st + 1] if st < NCH - 1 else ball
                        dsl = slice(hp * HP, (hp + 1) * HP)
                        if st == 0:
                            nc.vector.tensor_copy(dest[:, dsl], psU[:])
                        else:
                            nc.vector.tensor_add(dest[:, dsl], psU[:],
                                                 L_sb[st][:, dsl])

                # ---- collective: allgather block sums (trigger early) -----
                cc_in = drpool.tile([128, W], BF16)
                cc_out = drpool.tile([N_CORES, 128, W], BF16,
                                     addr_space="Shared")
                nc.sync.dma_start(cc_in[:], ball[:])
                nc.gpsimd.collective_compute(
                    "AllGather", Alu.bypass,
                    replica_groups=[list(range(N_CORES))],
                    ins=[cc_in[:]], outs=[cc_out[:]])

                # ---- qT projections + elu (overlap collective) ------------
                sqT = [dpool.tile([128, S_BLK], BF16, name=f"sqT{hp}")
                       for hp in range(NHP)]
                for hp in range(NHP):
                    psq = ps1.tile([128, S_BLK], F32, name="psbig", bufs=2)
                    for dt in range(4):
                        nc.tensor.matmul(
                            psq[:],
                            wq_t[dt][:, hp * 128:(hp + 1) * 128],
                            hsT[dt][:], start=(dt == 0), stop=(dt == 3))
                    e_t = tpool.tile([128, S_BLK], F32, name="elu_e")
                    r_t = tpool.tile([128, S_BLK], F32, name="elu_r")
                    nc.scalar.activation(e_t[:], psq[:], Act.Exp,
                                         bias=bqt[:, hp:hp + 1])
                    nc.scalar.activation(r_t[:], psq[:], Act.Relu,
                                         bias=bqt[:, hp:hp + 1])
                    nc.vector.scalar_tensor_tensor(
                        sqT[hp][:], e_t[:], 1.0, r_t[:],
                        op0=Alu.min, op1=Alu.add)

                # ---- pre-P pass: scores + causal mask for all chunks ------
                ni_sb = [[None] * NHP for _ in range(NCH)]
                with tc.tile_pool(name="am", bufs=1) as ampool:
                    # local-state seed G0_c = mz + L_c (P-independent)
                    G0 = [None] * NCH
                    g00 = dpool.tile([128, W], BF16, name="G0_0")
                    nc.vector.tensor_copy(g00[:], mz[:])
                    G0[0] = g00
                    for c in range(1, NCH):
                        g0 = dpool.tile([128, W], BF16, name=f"G0_{c}")
                        nc.vector.tensor_add(g0[:], mz[:], L_sb[c][:])
                        G0[c] = g0
                    for c in range(NCH):
                        cs = slice(c * 128, (c + 1) * 128)
                        for hp in range(NHP):
                            am = ampool.tile([128, 256], BF16,
                                             name=f"am{c}_{hp}", bufs=2,
                                             tag="am")
                            ni = ampool.tile([128, 2 * HP], BF16,
                                             name=f"ni{c}_{hp}")
                            for sub in range(2):
                                h = 2 * hp + sub
                                hb = slice(sub * 64, (sub + 1) * 64)
                                asl = slice(sub * 128, (sub + 1) * 128)
                                psA = ps1.tile([128, 128], F32, name="psa",
                                               bufs=2, padded_shape=[128, 512])
                                nc.tensor.matmul(
                                    psA[:], skT[hp][hb, cs], sqT[hp][hb, cs],
                                    start=True, stop=True)
                                nc.vector.tensor_mul(
                                    am[:, asl], psA[:], triu2[:, 0:128])
                                # intra-chunk numerator (P-independent)
                                psNi = ps1.tile([128, HP], F32, name="psu",
                                                bufs=2, padded_shape=[128, 512])
                                nc.tensor.matmul(
                                    psNi[:], am[:, asl],
                                    v_sb[c][:, h * HP:(h + 1) * HP],
                                    start=True, stop=False)
                                nc.tensor.matmul(
                                    psNi[:], sqT[hp][hb, cs],
                                    G0[c][hb, hp * HP:(hp + 1) * HP],
                                    start=False, stop=True)
                                nc.vector.tensor_copy(
                                    ni[:, sub * HP:(sub + 1) * HP], psNi[:])
                            ni_sb[c][hp] = ni

                    # ---- prefix state P = sum_{j<me} Bg_j ------------
                    bg = dpool.tile([128, N_CORES * W], BF16, name="bg")
                    nc.sync.dma_start(
                        bg.rearrange("p (j e) -> p j e", j=N_CORES),
                        cc_out.rearrange("j p e -> p j e"))
                    pa = dpool.tile([128, W], F32, name="pa")
                    pb = dpool.tile([128, W], F32, name="pb")
                    nc.vector.tensor_scalar_mul(pa[:], bg[:, 0:W],
                                                pmask[:, 0:1])
                    acc_src = pa
                    for j in range(1, N_CORES - 1):
                        acc_dst = pb if j % 2 == 1 else pa
                        nc.vector.scalar_tensor_tensor(
                            acc_dst[:], bg[:, j * W:(j + 1) * W],
                            pmask[:, j:j + 1], acc_src[:],
                            op0=Alu.mult, op1=Alu.add)
                        acc_src = acc_dst
                    PP = dpool.tile([128, W], BF16, name="PPb")
                    nc.vector.tensor_copy(PP[:], acc_src[:])

                    _ctx["ni_sb"] = ni_sb
                    _ctx["PP"] = PP

            ni_sb = _ctx["ni_sb"]
            PP = _ctx["PP"]
            # ---- post-P pass: inter term, divide, transpose ---------------
            attnT = [dpool.tile([128, S_BLK], BF16, name=f"attnT{hp}")
                     for hp in range(NHP)]
            with tc.tile_pool(name="ps2", bufs=1, space="PSUM") as ps2:
                for c in range(NCH):
                    cs = slice(c * 128, (c + 1) * 128)
                    for hp in range(NHP):
                        ap_ = tpool.tile([128, 128], BF16, name="attnp")
                        for sub in range(2):
                            hb = slice(sub * 64, (sub + 1) * 64)
                            nsl = slice(sub * HP, (sub + 1) * HP)
                            psN = ps2.tile([128, HP], F32, name="psN", bufs=3,
                                           padded_shape=[128, 512])
                            nc.tensor.matmul(
                                psN[:], identb[:],
                                ni_sb[c][hp][:, nsl],
                                start=True, stop=False)
                            nc.tensor.matmul(
                                psN[:], sqT[hp][hb, cs],
                                PP[hb, hp * HP:(hp + 1) * HP],
                                start=False, stop=True)
                            rec = spool.tile([128, 1], F32, name="rec")
                            nc.vector.reciprocal(rec[:], psN[:, DH:DH + 1])
                            nc.scalar.activation(
                                ap_[:, sub * DH:(sub + 1) * DH],
                                psN[:, 0:DH], Act.Copy, scale=rec[:])
                        psT = ps2.tile([128, 128], BF16, name="pstr2", bufs=2,
                                       padded_shape=[128, 1024])
                        nc.tensor.transpose(psT[:], ap_[:], identb[:])
                        nc.vector.tensor_copy(attnT[hp][:, cs], psT[:])

                # ---- output projection ------------------------------------
                for st in range(4):
                    ss = slice(st * 128, (st + 1) * 128)
                    psO = ps2.tile([128, D], F32, name="psO", bufs=2)
                    for hp in range(NHP):
                        nc.tensor.matmul(
                            psO[:], attnT[hp][:, ss], wo_t[hp][:],
                            start=(hp == 0), stop=(hp == NHP - 1))
                    y_sb = tpool.tile([128, D], F32, name="ysb", bufs=2)
                    nc.vector.tensor_copy(y_sb[:], psO[:])
                    nc.sync.dma_start(y_d[ss, :], y_sb[:])

    nc.compile()
    return nc


def _get_nc():
    if "nc" not in _CACHE:
        _CACHE["nc"] = _build()
    return _CACHE["nc"]


def _make_in_maps(hidden_states, Wq, bq, Wk, bk, Wv, bv, Wo, M_mem, z_mem):
    bf16 = ml_dtypes.bfloat16
    hs = np.asarray(hidden_states, np.float32).reshape(S, D)
    Wq = np.asarray(Wq, np.float32).astype(bf16)
    Wk = np.asarray(Wk, np.float32).astype(bf16)
    Wv = np.asarray(Wv, np.float32).astype(bf16)
    Wo = np.asarray(Wo, np.float32).astype(bf16)
    bq = np.asarray(bq, np.float32)
    bk = np.asarray(bk, np.float32)
    bv = np.asarray(bv, np.float32)
    M_mem = np.asarray(M_mem, np.float32)
    z_mem = np.asarray(z_mem, np.float32)

    bqt = np.ascontiguousarray(bq.reshape(NHP, 128).T)
    bkt = np.ascontiguousarray(bk.reshape(NHP, 128).T)

    mz = np.zeros((128, W), np.float32)
    for h in range(H):
        pr, col = (h % 2) * 64, (h // 2) * HP
        mz[pr:pr + 64, col:col + DH] = M_mem[h]
        mz[pr:pr + 64, col + DH] = z_mem[h]

    in_maps = []
    for c in range(N_CORES):
        pm = np.zeros((128, N_CORES), np.float32)
        pm[:, :c] = 1.0
        hst = np.ascontiguousarray(
            hs[c * S_BLK:(c + 1) * S_BLK].T).astype(bf16)
        in_maps.append({
            "hst": hst,
            "wq": Wq, "wk": Wk, "wv": Wv, "wo": Wo,
            "bqt": bqt, "bkt": bkt,
            "bvr": bv.astype(bf16).reshape(1, D),
            "mz": mz, "pmask": pm,
        })
    return in_maps


def kernel(hidden_states, Wq, bq, Wk, bk, Wv, bv, Wo, M_mem, z_mem):
    nc = _get_nc()
    in_maps = _make_in_maps(hidden_states, Wq, bq, Wk, bk, Wv, bv, Wo,
                            M_mem, z_mem)
    res = run_bass_kernel_spmd(nc, in_maps, list(range(N_CORES)))
    out = np.concatenate([res.results[c]["y"] for c in range(N_CORES)], axis=0)
    return out.reshape(1, S, D)
